# revision 7
# baseline (speedup 1.0000x reference)
"""Trainium2 Bass kernel for triangle (AlphaFold-style) gated attention over pair rows.

Problem: B=1, N=256 rows; per row n: attention over 256 positions,
H=4 heads x CH=32, C=128 channels, additive mask bias (per row, per key),
triangle bias (per head, q, k; shared across rows), sigmoid gating,
output projection. Rows sharded across 8 NeuronCores (32 rows/core), SPMD.

v3 dataflow (vs the v2 baseline: no PE transposes, no M-precompute/u-stage,
heavy use of 32-strip PE tiling for 4-way concurrent matmuls):
  - host pre-transposes + casts inputs: xqT/xkT [c=128, tok=256] fp16 staged
    in DRAM, DMA'd straight to SBUF (kills PE transposes + gpsimd cast DMAs)
  - projections qp_h = (wq_h*scale*256) @ xqT, kp_h = wk_h @ xkT as two
    4-way col-tiled PE batches -> psQP [128=(h,d), q|k], one DVE cast to fp16
  - scores sT_h[k,q] = kp_h.T @ qp_h as K=32 row-packed MMs, 4 heads
    concurrent per k-tile; triangle bias (x256, fp16) accumulated by 4-way
    col-tiled identity matmuls (tri shared across rows)
  - p = exp(psS/256 + mask) one ACT op per k-tile [128,1024], mask is the
    per-partition bias; exp(-1e9)=0 reproduces the reference mask exactly
  - oT[hd,q] and broadcast denominators (sele=2.0, folds the tanh-form
    sigmoid's 0.5) via 4-way col-tiled MMs accumulating over k-tiles
  - gating tanh on ACT (same table set as exp); g2=(1+tanh)*recip(den) built
    on GPSIMD (two fp16 SBUF tensor_tensor ops) to offload DVE
  - out[q,c] = (oT*g2) @ wo.T + 1 x bo, stored fp16, host casts to fp32
"""
import numpy as np

B, N, CQ, H, CH = 1, 256, 128, 4, 32
NCORES = 8
ROWS = N // NCORES  # 32
HD = H * CH  # 128


def build_program(rows):
    import concourse.bass as bass
    import concourse.bacc as bacc
    import concourse.mybir as mybir
    from concourse import tile

    f32 = mybir.dt.float32
    fp16 = mybir.dt.float16
    AF = mybir.ActivationFunctionType
    nc = bacc.Bacc("TRN2", target_bir_lowering=False, debug=False)

    qxT = nc.declare_dram_parameter("qxT", [rows, CQ, N], fp16, isOutput=False)
    kvT = nc.declare_dram_parameter("kvT", [rows, CQ, N], fp16, isOutput=False)
    maskc = nc.declare_dram_parameter("maskc", [128, rows, 2], f32, isOutput=False)
    triT = nc.declare_dram_parameter("triT", [4, 128, 512], fp16, isOutput=False)
    mcat = nc.declare_dram_parameter("mcat", [CQ, H * CQ], fp16, isOutput=False)
    wvT = nc.declare_dram_parameter("wvT", [CQ, HD], fp16, isOutput=False)
    wgT = nc.declare_dram_parameter("wgT", [CQ, HD], fp16, isOutput=False)
    woT = nc.declare_dram_parameter("woT", [HD, CQ], fp16, isOutput=False)
    bgc = nc.declare_dram_parameter("bgc", [HD, 1], f32, isOutput=False)
    bor = nc.declare_dram_parameter("bor", [1, CQ], fp16, isOutput=False)
    onesr = nc.declare_dram_parameter("onesr", [1, 128], fp16, isOutput=False)
    sele = nc.declare_dram_parameter("sele", [128, 32], fp16, isOutput=False)
    id16 = nc.declare_dram_parameter("id16", [128, 128], fp16, isOutput=False)
    out = nc.declare_dram_parameter("out", [rows, N, CQ], fp16, isOutput=True)

    with tile.TileContext(nc) as tc:
        with (
            nc.allow_low_precision(reason="fp16 matmul operands and "
                                   "reciprocal_approx_fast by design"),
            tc.tile_pool(name="const", bufs=1) as cp,
            tc.tile_pool(name="sx", bufs=3) as sx,
            tc.tile_pool(name="sb", bufs=2) as sb,
            tc.tile_pool(name="ps", bufs=2, space=bass.MemorySpace.PSUM) as ps,
            tc.tile_pool(name="ps1", bufs=1, space=bass.MemorySpace.PSUM) as ps1,
        ):
            # ---- constants ----
            m_s = cp.tile([CQ, H * CQ], fp16, tag="mcat")
            wv_s = cp.tile([CQ, HD], fp16, tag="wv")
            wg_s = cp.tile([CQ, HD], fp16, tag="wg")
            wo_s = cp.tile([HD, CQ], fp16, tag="wo")
            bg_s = cp.tile([HD, 1], f32, tag="bg")
            bo_s = cp.tile([1, CQ], fp16, tag="bo")
            ones_r = cp.tile([1, 128], fp16, tag="onr")
            sel_s = cp.tile([128, 32], fp16, tag="sele")
            id_s = cp.tile([128, 128], fp16, tag="id")
            tri_s = cp.tile([128, 4 * 512], fp16, tag="tri")
            mk_all = cp.tile([128, rows, 2], f32, tag="mkall")
            for t, d in ((m_s, mcat), (wv_s, wvT), (wg_s, wgT),
                         (wo_s, woT), (bg_s, bgc), (bo_s, bor), (ones_r, onesr),
                         (sel_s, sele), (id_s, id16), (mk_all, maskc)):
                nc.sync.dma_start(t[:], d[:])
            for i in range(4):
                nc.sync.dma_start(tri_s[:, i * 512:(i + 1) * 512], triT[i])

            for n in range(rows):
                # ---- input loads (pre-transposed fp16 from host) ----
                xqT = sx.tile([CQ, N], fp16, tag="xqT")
                xkT = sx.tile([CQ, N], fp16, tag="xkT")
                nc.sync.dma_start(xqT[:], qxT[n])
                nc.sync.dma_start(xkT[:], kvT[n])

                # ---- scores stage 1: u_h = (wk_h.T wq_h * scale*256).T @ xkT ----
                psU = ps1.tile([128, H * N], f32, tag="psU")
                for h in range(H):
                    nc.tensor.matmul(psU[:, h * N:(h + 1) * N],
                                     m_s[:, h * CQ:(h + 1) * CQ], xkT[:],
                                     start=True, stop=True)
                u16 = sb.tile([128, H * N], fp16, tag="u16")
                nc.vector.tensor_copy(u16[:], psU[:])

                # ---- v and gating projections ----
                psBG = ps1.tile([128, 2 * N], f32, tag="psBG")  # v(2 tok-tiles) | gT
                nc.tensor.matmul(psBG[:, 0:128], xkT[:, 0:128], wv_s[:],
                                 start=True, stop=True)
                nc.tensor.matmul(psBG[:, 128:N], xkT[:, 128:N], wv_s[:],
                                 start=True, stop=True)
                nc.tensor.matmul(psBG[:, N:2 * N], wg_s[:], xqT[:],
                                 start=True, stop=True)
                v16 = sb.tile([128, N], fp16, tag="v16")
                nc.vector.tensor_copy(v16[:], psBG[:, 0:N])
                # gating via tanh (same ACT table set as exp):
                # sigmoid(x) = 0.5*(1+tanh(x/2)); the 0.5 is folded into sele=2
                tT = sb.tile([128, N], f32, tag="tT")
                nc.scalar.activation(tT[:], psBG[:, N:2 * N], AF.Tanh,
                                     scale=0.5, bias=bg_s[:, 0:1])

                # ---- scores (K=32 row-packed) + triangle + exp ----
                pT = []
                for kt in range(2):
                    psS = ps.tile([128, H * N], f32, tag="psS")
                    # triangle bias first (start=True sets has_written for the
                    # whole bank per written partition-rows; scores then
                    # accumulate with start=False -- order matters, a later
                    # start=True would clear earlier MMs' has_written bits)
                    for half in range(2):  # head pair (bank) halves
                        for g in range(4):
                            nc.tensor.matmul(
                                psS[32 * g:32 * g + 32,
                                    half * 512:half * 512 + 512],
                                id_s[:, 32 * g:32 * g + 32],
                                tri_s[:, (2 * kt + half) * 512:
                                      (2 * kt + half) * 512 + 512],
                                start=True, stop=False,
                                tile_position=(0, 32 * g),
                                skip_group_check=True)
                    for h in range(H):
                        nc.tensor.matmul(
                            psS[:, h * N:(h + 1) * N],
                            u16[:, h * N + kt * 128:h * N + kt * 128 + 128],
                            xqT[:],
                            start=False, stop=(h % 2 == 1))
                    pTk = sb.tile([128, H * N], fp16, tag=f"pT{kt}")
                    nc.scalar.activation(pTk[:], psS[:], AF.Exp,
                                         scale=float(1.0 / 256.0),
                                         bias=mk_all[:, n, kt:kt + 1])
                    pT.append(pTk)

                # ---- AV (oT) + broadcast denominators, 4-way col-tiled ----
                psOD = ps1.tile([128, 2 * N], f32, tag="psOD")  # oT | den
                for kt in range(2):
                    for h in range(H):
                        nc.tensor.matmul(
                            psOD[32 * h:32 * h + 32, 0:N],
                            v16[:, kt * 128 + 32 * h:kt * 128 + 32 * h + 32],
                            pT[kt][:, h * N:(h + 1) * N],
                            start=(kt == 0), stop=(kt == 1),
                            tile_position=(0, 32 * h), skip_group_check=True)
                for kt in range(2):
                    for h in range(H):
                        nc.tensor.matmul(
                            psOD[32 * h:32 * h + 32, N:2 * N], sel_s[:],
                            pT[kt][:, h * N:(h + 1) * N],
                            start=(kt == 0), stop=(kt == 1),
                            tile_position=(0, 32 * h), skip_group_check=True)
                rb_s = sb.tile([128, N], f32, tag="rb")
                nc.vector.reciprocal_approx_fast(rb_s[:], psOD[:, N:2 * N])

                # ---- g2 = (1+tanh)*rb on gpsimd (frees DVE) ----
                t1 = sb.tile([128, N], f32, tag="t1")
                nc.vector.tensor_mul(t1[:], tT[:], rb_s[:])
                g2 = sb.tile([128, N], f32, tag="g2")
                nc.vector.tensor_add(g2[:], t1[:], rb_s[:])

                # ---- gate*normalize, final projection (natural out) ----
                og2 = sb.tile([128, N], fp16, tag="og2")
                nc.vector.tensor_mul(og2[:], psOD[:, 0:N], g2[:])
                # final projection writes back into the freed psOD o-half
                for qt in range(2):
                    nc.tensor.matmul(psOD[:, qt * 128:(qt + 1) * 128],
                                     og2[:, qt * 128:(qt + 1) * 128], wo_s[:],
                                     start=True, stop=False)
                    nc.tensor.matmul(psOD[:, qt * 128:(qt + 1) * 128],
                                     ones_r[:], bo_s[:], start=False,
                                     stop=True)
                o16 = sb.tile([128, N], fp16, tag="o16")
                nc.vector.tensor_copy(o16[:], psOD[:, 0:N])
                for qt in range(2):
                    nc.sync.dma_start(out[n, qt * 128:(qt + 1) * 128, :],
                                      o16[:, qt * 128:(qt + 1) * 128])
    nc.compile()
    return nc


_PROG_CACHE = {}


def host_prep(q_x, kv_x, mask_bias, triangle_bias, wq, wk, wv, wg, bg, wo, bo):
    scale = np.float64(1.0 / np.float64(np.sqrt(np.float32(CH), dtype=np.float32)))
    qxT = np.ascontiguousarray(
        np.asarray(q_x, np.float32).reshape(N, N, CQ).transpose(0, 2, 1)
        .astype(np.float16))  # [n, c, q]
    kvT = np.ascontiguousarray(
        np.asarray(kv_x, np.float32).reshape(N, N, CQ).transpose(0, 2, 1)
        .astype(np.float16))  # [n, c, k]

    # M_h = wk_h.T @ wq_h * scale * 256 (x256 dodges fp16 subnormals;
    # exp's scale=1/256 compensates), mcat [c, h*CQ + c']
    wqf = np.asarray(wq, np.float64).reshape(H, CH, CQ)
    wkf = np.asarray(wk, np.float64).reshape(H, CH, CQ)
    mcat = np.ascontiguousarray(np.concatenate(
        [(wkf[h].T @ wqf[h] * (scale * 256.0)) for h in range(H)],
        axis=1).astype(np.float16))
    wvT = np.ascontiguousarray(np.asarray(wv).reshape(HD, CQ).T.astype(np.float16))
    wgT = np.ascontiguousarray(np.asarray(wg).reshape(HD, CQ).T.astype(np.float16))
    woT = np.ascontiguousarray(np.asarray(wo).T.astype(np.float16))  # [e, c]
    bgc = np.ascontiguousarray(np.asarray(bg, np.float32).reshape(HD, 1) * 0.5)
    bor = np.ascontiguousarray(np.asarray(bo).reshape(1, CQ).astype(np.float16))
    onesr = np.ones((1, 128), np.float16)
    sele = np.full((128, 32), 2.0, np.float16)
    id16 = np.eye(128, dtype=np.float16)
    # mask: [n, k] -> [k_in_tile, n, kt] (per-partition exp bias)
    m = np.asarray(mask_bias, np.float32).reshape(N, N)
    maskc = np.ascontiguousarray(m.reshape(N, 2, 128).transpose(2, 0, 1))
    # triangle x256: [h, q, k] -> [(kt, half), k_in_tile, (h2, q)] where
    # half selects head pair (h2 in {0,1} within), matching psS col layout
    t = np.asarray(triangle_bias, np.float64).reshape(H, N, N) * 256.0
    tT = t.transpose(0, 2, 1).reshape(H, 2, 128, N)  # [h, kt, kin, q]
    triT = np.ascontiguousarray(
        tT.transpose(1, 0, 2, 3).reshape(2, 2, 2, 128, N)  # [kt, half, h2, kin, q]
        .transpose(0, 1, 3, 2, 4).reshape(4, 128, 512).astype(np.float16))
    shared = dict(mcat=mcat, wvT=wvT, wgT=wgT, woT=woT, bgc=bgc,
                  bor=bor, onesr=onesr, sele=sele, id16=id16, triT=triT)
    return qxT, kvT, maskc, shared


def make_in_maps(q_x, kv_x, mask_bias, triangle_bias, wq, wk, wv, wg, bg, wo, bo):
    qxT, kvT, maskc, shared = host_prep(q_x, kv_x, mask_bias, triangle_bias,
                                        wq, wk, wv, wg, bg, wo, bo)
    in_maps = []
    for i in range(NCORES):
        sl = slice(i * ROWS, (i + 1) * ROWS)
        in_maps.append(dict(qxT=np.ascontiguousarray(qxT[sl]),
                            kvT=np.ascontiguousarray(kvT[sl]),
                            maskc=np.ascontiguousarray(maskc[:, sl]), **shared))
    return in_maps


def get_program():
    if ROWS not in _PROG_CACHE:
        _PROG_CACHE[ROWS] = build_program(ROWS)
    return _PROG_CACHE[ROWS]


def kernel(q_x, kv_x, mask_bias, triangle_bias, wq, wk, wv, wg, bg, wo, bo):
    from concourse.bass_utils import run_bass_kernel_spmd

    in_maps = make_in_maps(q_x, kv_x, mask_bias, triangle_bias,
                           wq, wk, wv, wg, bg, wo, bo)
    nc = get_program()
    res = run_bass_kernel_spmd(nc, in_maps, list(range(NCORES)))
    outs = [np.asarray(res.results[i]["out"]) for i in range(NCORES)]
    return np.concatenate(outs, axis=0)[None].astype(np.float32)
